# revision 1
# baseline (speedup 1.0000x reference)
"""DeepSeek-V4 MLA sparse attention — Trainium2 Bass kernel, 8 NeuronCores.

Contract: kernel(**inputs) takes the FULL unsharded inputs
  q [512,64,576] f32, kv_cache [32768,576] f32,
  topk_indices [512,512] i32, attn_sink [64] f32
and returns the FULL output [512,64,512] f32.

Strategy (token/data-parallel per the sharding hint):
  - tokens sharded 8 ways (64/core); kv_cache replicated per core.
  - host prep: q scaled by 576^-0.5 and laid out d-major in 128-partition
    chunks [t,128p,5c,64h]; topk -> int16 in the SWDGE 16-partition wrap
    (idx j at partition j%16, replicated x8 for the Q7 cores);
    exp(attn_sink) precomputed; identity matrix for PE transposes.
  - device, per token pair (A,B):
      * gpsimd dma_gather: 512 rows x 2304B fp32 from the cache
        -> SBUF [128 (j%128), 4 (j//128), 576]; the first 512 columns of
        the gathered rows double as V for the PV matmul (topk-major).
      * PE transpose-mode matmuls build K^T [128d, 5c, 512j]; ScalarE
        drains PSUM -> SBUF.
      * QK^T: matmuls column-tiled across the PE array (token A -> array
        cols / psum partitions 0-63, token B -> 64-127), accumulating over
        the 5 d-chunks (last chunk K=64).
      * sink-softmax with NO max-subtraction (shift-invariant; logits are
        ~N(0,1) by construction so exp cannot overflow fp32):
        p = exp(s); denom = sum(p) + exp(sink). exp + row-sum fused in one
        ScalarE activation; denom/recip on DVE.
      * p^T via one [128,128] PE transpose per topk block (both tokens at
        once thanks to the column-tiled layout).
      * PV: matmuls column-tiled over 4 topk blocks; out = pv * (1/denom)
        fused with the PSUM->SBUF drain on DVE; one DMA stores both tokens.

MODE selects matmul precision:
  "fp32"  all-fp32 (PE 4 cycles/row)                      rel err ~3e-6
  "pv_rx" PV in fp32r (11-bit mantissa) with V rounded
          on-device; QK/scores stay exact fp32            rel err ~1.5e-5
  "pv_r"  KV cache host-rounded to fp32r; transposes+PV
          fp32r; QK fp32 over rounded K                   rel err ~1e-4
  "all_r" pv_r plus QK in fp32r                           rel err ~2e-4
"""

import numpy as np
from contextlib import ExitStack, nullcontext

import concourse.mybir as mybir
import concourse.tile as tile
from concourse import bacc
from concourse.bass_utils import run_bass_kernel_spmd

F32 = mybir.dt.float32
F32R = mybir.dt.float32r
BF16 = mybir.dt.bfloat16
I16 = mybir.dt.int16

T_FULL = 512
H = 64
D = 576
DV = 512
NKV = 32768
TOPK = 512
N_CORES = 8
T_LOC = T_FULL // N_CORES
SCALE = float(D) ** -0.5
NCH = 5   # ceil(576/128) d-chunks
NB = TOPK // 128  # topk blocks of 128

MODE = "fp32"  # set from measurement; see module docstring


def build_program(t_loc=T_LOC, repeat=1, mode=MODE):
    assert mode in ("fp32", "pv_r", "pv_rx", "all_r", "bf3")
    if mode == "bf3":
        return build_program_bf3(t_loc, repeat)
    rkv = mode in ("pv_r", "all_r")
    rqk = mode == "all_r"
    vx = mode == "pv_rx"
    rpv = mode != "fp32"
    KD = F32R if rkv else F32
    QD = F32R if rqk else F32
    PD = F32R if rqk else F32
    TD = F32R if rpv else F32

    nc = bacc.Bacc("TRN2", target_bir_lowering=False, debug=False)
    q_t = nc.dram_tensor("q_t", [t_loc, 128, NCH, H], F32, kind="ExternalInput")
    kv = nc.dram_tensor("kv", [NKV, D], KD, kind="ExternalInput")
    idx = nc.dram_tensor("idx", [t_loc, 128, TOPK // 16], I16,
                         kind="ExternalInput")
    esink = nc.dram_tensor("esink", [128, 1], F32, kind="ExternalInput")
    ident_d = nc.dram_tensor("ident", [128, 128], F32, kind="ExternalInput")
    out = nc.dram_tensor("out", [t_loc, H, DV], F32, kind="ExternalOutput")

    out_flat = out.ap().rearrange("t h d -> (t h) d")

    with tile.TileContext(nc) as tc, ExitStack() as ctx:
        consts = ctx.enter_context(tc.tile_pool(name="consts", bufs=1))
        kq = ctx.enter_context(tc.tile_pool(name="kq", bufs=5))
        ktp = ctx.enter_context(tc.tile_pool(name="ktp", bufs=3))
        soft = ctx.enter_context(tc.tile_pool(name="soft", bufs=2))
        outp = ctx.enter_context(tc.tile_pool(name="outp", bufs=2))
        small = ctx.enter_context(tc.tile_pool(name="small", bufs=4))
        ps_kt = ctx.enter_context(
            tc.tile_pool(name="ps_kt", bufs=2, space="PSUM"))
        ps_sc = ctx.enter_context(
            tc.tile_pool(name="ps_sc", bufs=2, space="PSUM"))
        ps_pt = ctx.enter_context(
            tc.tile_pool(name="ps_pt", bufs=2, space="PSUM"))
        ps_pv = ctx.enter_context(
            tc.tile_pool(name="ps_pv", bufs=2, space="PSUM"))

        ident = consts.tile([128, 128], F32)
        nc.sync.dma_start(out=ident[:], in_=ident_d.ap())
        es_sb = consts.tile([128, 1], F32)
        nc.sync.dma_start(out=es_sb[:], in_=esink.ap())
        if rkv:
            identk = consts.tile([128, 128], KD)
            nc.scalar.copy(identk[:], ident[:])  # 0/1 exact on the fp32r grid
        else:
            identk = ident
        identp = identk if rqk else ident
        # Warmup transpose absorbs the identity-DMA wait up front.
        warm = ps_pt.tile([128, 128], F32, tag="ps_pt")
        nc.tensor.transpose(warm[:], ident[:], ident[:])

        def load_token(t):
            idx_sb = kq.tile([128, TOPK // 16], I16, tag="idx")
            nc.sync.dma_start(out=idx_sb[:], in_=idx.ap()[t])
            k_sb = kq.tile([128, NB, D], KD, tag="k")
            nc.gpsimd.dma_gather(
                out_ap=k_sb[:],
                in_ap=kv.ap(),
                idxs_ap=idx_sb[:],
                num_idxs=TOPK,
                num_idxs_reg=TOPK,
                elem_size=D,
            )
            q_sb = kq.tile([128, NCH, H], F32, tag="q")
            nc.sync.dma_start(out=q_sb[:], in_=q_t.ap()[t])
            # Funnel q through ScalarE: single upstream semaphore for QK
            # (keeps per-instruction wait counts legal) and, in all_r, the
            # fp32r rounding point for q.
            q_act = kq.tile([128, NCH, H], QD, tag="qa")
            nc.scalar.copy(q_act[:], q_sb[:])
            if vx:
                v_r = kq.tile([128, NB, DV], F32R, tag="vr")
                nc.scalar.copy(v_r[:], k_sb[:, :, 0:DV])
            else:
                v_r = k_sb
            return k_sb, q_act, v_r

        def build_kt(k_sb):
            kt_sb = ktp.tile([128, NCH, TOPK], QD, tag="kt")
            for c in range(NCH):
                pp = 128 if c < 4 else D - 512
                pst = ps_kt.tile([128, TOPK], KD, tag="ps_kt")
                for b in range(NB):
                    nc.tensor.transpose(
                        pst[:pp, b * 128:(b + 1) * 128],
                        k_sb[:, b, c * 128:c * 128 + pp],
                        identk[:],
                    )
                nc.scalar.copy(kt_sb[:pp, c, :], pst[:pp, :])
            return kt_sb

        def pair_body(tA):
            kA, qA, vA = load_token(tA)
            kB, qB, vB = load_token(tA + 1)
            ktA = build_kt(kA)
            ktB = build_kt(kB)

            sc = ps_sc.tile([128, TOPK], F32, tag="sc")
            for c in range(NCH):
                kk = 128 if c < 4 else D - 512
                st, sp = (c == 0), (c == NCH - 1)
                nc.tensor.matmul(
                    sc[0:64, :], lhsT=qA[:kk, c, :], rhs=ktA[:kk, c, :],
                    start=st, stop=sp, tile_position=(0, 0),
                    skip_group_check=True,
                )
                nc.tensor.matmul(
                    sc[64:128, :], lhsT=qB[:kk, c, :], rhs=ktB[:kk, c, :],
                    start=st, stop=sp, tile_position=(0, 64),
                    skip_group_check=True,
                )

            p_sb = soft.tile([128, TOPK], PD, tag="p")
            sum_p = small.tile([128, 1], F32, tag="sum")
            nc.scalar.activation(
                p_sb[:], sc[:], mybir.ActivationFunctionType.Exp,
                accum_out=sum_p[:],
            )
            den = small.tile([128, 1], F32, tag="den")
            nc.vector.tensor_add(den[:], sum_p[:], es_sb[:])
            rec = small.tile([128, 1], F32, tag="rec")
            nc.vector.reciprocal(rec[:], den[:])

            pt_sb = soft.tile([128, NB, 128], TD, tag="pt")
            for b in range(NB):
                pst = ps_pt.tile([128, 128], PD, tag="ps_pt")
                nc.tensor.transpose(
                    pst[:], p_sb[:, b * 128:(b + 1) * 128], identp[:])
                nc.vector.tensor_copy(pt_sb[:, b, :], pst[:])

            pv = ps_pv.tile([128, DV], F32, tag="pv")
            for b in range(NB):
                st, sp = (b == 0), (b == NB - 1)
                nc.tensor.matmul(
                    pv[0:64, :], lhsT=pt_sb[:, b, 0:64],
                    rhs=vA[:, b, 0:DV] if not vx else vA[:, b, :],
                    start=st, stop=sp, tile_position=(0, 0),
                    skip_group_check=True,
                )
                nc.tensor.matmul(
                    pv[64:128, :], lhsT=pt_sb[:, b, 64:128],
                    rhs=vB[:, b, 0:DV] if not vx else vB[:, b, :],
                    start=st, stop=sp, tile_position=(0, 64),
                    skip_group_check=True,
                )

            o_sb = outp.tile([128, DV], F32, tag="o")
            nc.vector.tensor_scalar_mul(o_sb[:], pv[:], rec[:])
            nc.sync.dma_start(
                out=out_flat[tA * H:tA * H + 128, :], in_=o_sb[:])

        loop_cm = tc.For_i(0, repeat, 1) if repeat > 1 else nullcontext()
        with loop_cm:
            for i in range(t_loc // 2):
                pair_body(2 * i)

    nc.compile()
    return nc


def build_program_bf3(t_loc=T_LOC, repeat=1):
    """3-pass bf16 hi/lo split matmuls: fp32-gathered K, fp32 PE transposes;
    QK/PV computed as hi*hi + hi*lo + lo*hi of bf16 splits (~17-bit operand
    precision, 3/4 the fp32 PE cost). q split on host; K^T/V/p splits fold
    into the PSUM drains (ScalarE copy -> hi, DVE subtract -> lo)."""
    nc = bacc.Bacc("TRN2", target_bir_lowering=False, debug=False)
    q_t = nc.dram_tensor("q_t", [t_loc, 128, NCH, 2, H], BF16,
                         kind="ExternalInput")
    kv = nc.dram_tensor("kv", [NKV, D], F32, kind="ExternalInput")
    idx = nc.dram_tensor("idx", [t_loc, 128, TOPK // 16], I16,
                         kind="ExternalInput")
    esink = nc.dram_tensor("esink", [128, 1], F32, kind="ExternalInput")
    ident_d = nc.dram_tensor("ident", [128, 128], F32, kind="ExternalInput")
    out = nc.dram_tensor("out", [t_loc, H, DV], F32, kind="ExternalOutput")

    out_flat = out.ap().rearrange("t h d -> (t h) d")

    with tile.TileContext(nc) as tc, ExitStack() as ctx:
        consts = ctx.enter_context(tc.tile_pool(name="consts", bufs=1))
        kq = ctx.enter_context(tc.tile_pool(name="kq", bufs=4))
        ktp = ctx.enter_context(tc.tile_pool(name="ktp", bufs=2))
        soft = ctx.enter_context(tc.tile_pool(name="soft", bufs=2))
        outp = ctx.enter_context(tc.tile_pool(name="outp", bufs=2))
        small = ctx.enter_context(tc.tile_pool(name="small", bufs=4))
        ps_kt = ctx.enter_context(
            tc.tile_pool(name="ps_kt", bufs=2, space="PSUM"))
        ps_sc = ctx.enter_context(
            tc.tile_pool(name="ps_sc", bufs=1, space="PSUM"))
        ps_pt = ctx.enter_context(
            tc.tile_pool(name="ps_pt", bufs=4, space="PSUM"))
        ps_pv = ctx.enter_context(
            tc.tile_pool(name="ps_pv", bufs=1, space="PSUM"))

        ident = consts.tile([128, 128], F32)
        nc.sync.dma_start(out=ident[:], in_=ident_d.ap())
        es_sb = consts.tile([128, 1], F32)
        nc.sync.dma_start(out=es_sb[:], in_=esink.ap())
        warm = ps_pt.tile([128, 128], F32, tag="ps_pt")
        nc.tensor.transpose(warm[:], ident[:], ident[:])

        def load_token(t):
            idx_sb = kq.tile([128, TOPK // 16], I16, tag="idx")
            nc.sync.dma_start(out=idx_sb[:], in_=idx.ap()[t])
            k_sb = kq.tile([128, NB, D], F32, tag="k")
            nc.gpsimd.dma_gather(
                out_ap=k_sb[:], in_ap=kv.ap(), idxs_ap=idx_sb[:],
                num_idxs=TOPK, num_idxs_reg=TOPK, elem_size=D,
            )
            q_sb = kq.tile([128, NCH, 2, H], BF16, tag="q")
            nc.sync.dma_start(out=q_sb[:], in_=q_t.ap()[t])
            q_act = kq.tile([128, NCH, 2, H], BF16, tag="qa")
            nc.scalar.copy(q_act[:], q_sb[:])
            # V hi/lo splits (topk-major, straight from the gathered rows)
            v_hi = kq.tile([128, NB, DV], BF16, tag="vh")
            nc.scalar.copy(v_hi[:], k_sb[:, :, 0:DV])
            v_lo = kq.tile([128, NB, DV], BF16, tag="vl")
            nc.vector.tensor_sub(v_lo[:], k_sb[:, :, 0:DV], v_hi[:])
            return k_sb, q_act, v_hi, v_lo

        def build_kt(k_sb):
            kt_hi = ktp.tile([128, NCH, TOPK], BF16, tag="kth")
            kt_lo = ktp.tile([128, NCH, TOPK], BF16, tag="ktl")
            for c in range(NCH):
                pp = 128 if c < 4 else D - 512
                pst = ps_kt.tile([128, TOPK], F32, tag="ps_kt")
                for b in range(NB):
                    nc.tensor.transpose(
                        pst[:pp, b * 128:(b + 1) * 128],
                        k_sb[:, b, c * 128:c * 128 + pp],
                        ident[:],
                    )
                nc.scalar.copy(kt_hi[:pp, c, :], pst[:pp, :])
                nc.vector.tensor_sub(
                    kt_lo[:pp, c, :], pst[:pp, :], kt_hi[:pp, c, :])
            return kt_hi, kt_lo

        def pair_body(tA):
            kA, qA, vAh, vAl = load_token(tA)
            kB, qB, vBh, vBl = load_token(tA + 1)
            ktAh, ktAl = build_kt(kA)
            ktBh, ktBl = build_kt(kB)

            sc = ps_sc.tile([128, TOPK], F32, tag="sc")
            first, last = (0, 0), (NCH - 1, 2)
            for c in range(NCH):
                kk = 128 if c < 4 else D - 512
                for p_i, (qs, kts_A, kts_B) in enumerate(
                        ((0, ktAh, ktBh), (0, ktAl, ktBl), (1, ktAh, ktBh))):
                    st = (c, p_i) == first
                    sp = (c, p_i) == last
                    nc.tensor.matmul(
                        sc[0:64, :], lhsT=qA[:kk, c, qs, :],
                        rhs=kts_A[:kk, c, :],
                        start=st, stop=sp, tile_position=(0, 0),
                        skip_group_check=True,
                    )
                    nc.tensor.matmul(
                        sc[64:128, :], lhsT=qB[:kk, c, qs, :],
                        rhs=kts_B[:kk, c, :],
                        start=st, stop=sp, tile_position=(0, 64),
                        skip_group_check=True,
                    )

            p_sb = soft.tile([128, TOPK], F32, tag="p")
            sum_p = small.tile([128, 1], F32, tag="sum")
            nc.scalar.activation(
                p_sb[:], sc[:], mybir.ActivationFunctionType.Exp,
                accum_out=sum_p[:],
            )
            den = small.tile([128, 1], F32, tag="den")
            nc.vector.tensor_add(den[:], sum_p[:], es_sb[:])
            rec = small.tile([128, 1], F32, tag="rec")
            nc.vector.reciprocal(rec[:], den[:])

            pt_hi = soft.tile([128, NB, 128], BF16, tag="pth")
            pt_lo = soft.tile([128, NB, 128], BF16, tag="ptl")
            for b in range(NB):
                pst = ps_pt.tile([128, 128], F32, tag="ps_pt")
                nc.tensor.transpose(
                    pst[:], p_sb[:, b * 128:(b + 1) * 128], ident[:])
                nc.vector.tensor_copy(pt_hi[:, b, :], pst[:])
                nc.vector.tensor_sub(pt_lo[:, b, :], pst[:], pt_hi[:, b, :])

            pv = ps_pv.tile([128, DV], F32, tag="pv")
            firstb, lastb = (0, 0), (NB - 1, 2)
            for b in range(NB):
                for p_i, (ptx, vxA, vxB) in enumerate(
                        ((pt_hi, vAh, vBh), (pt_hi, vAl, vBl),
                         (pt_lo, vAh, vBh))):
                    st = (b, p_i) == firstb
                    sp = (b, p_i) == lastb
                    nc.tensor.matmul(
                        pv[0:64, :], lhsT=ptx[:, b, 0:64], rhs=vxA[:, b, :],
                        start=st, stop=sp, tile_position=(0, 0),
                        skip_group_check=True,
                    )
                    nc.tensor.matmul(
                        pv[64:128, :], lhsT=ptx[:, b, 64:128],
                        rhs=vxB[:, b, :],
                        start=st, stop=sp, tile_position=(0, 64),
                        skip_group_check=True,
                    )

            o_sb = outp.tile([128, DV], F32, tag="o")
            nc.vector.tensor_scalar_mul(o_sb[:], pv[:], rec[:])
            nc.sync.dma_start(
                out=out_flat[tA * H:tA * H + 128, :], in_=o_sb[:])

        loop_cm = tc.For_i(0, repeat, 1) if repeat > 1 else nullcontext()
        with loop_cm:
            for i in range(t_loc // 2):
                pair_body(2 * i)

    nc.compile()
    return nc


# ---------------- host-side prep ----------------

def round_f32r(x):
    """Round fp32 array to the fp32r grid (11 mantissa bits, RNE)."""
    u = np.ascontiguousarray(x, np.float32).view(np.uint32).astype(np.uint64)
    sh = 12
    r = (u + 0x7FF + ((u >> sh) & 1)) >> sh << sh
    return (r & 0xFFFFFFFF).astype(np.uint32).view(np.float32)


def prep_core_inputs(q, kv_rep, topk_indices, esink, ident, core,
                     t_loc=T_LOC, mode=MODE):
    t0 = core * t_loc
    qs = (np.asarray(q[t0:t0 + t_loc]) * SCALE).astype(np.float32)
    qpad = np.zeros((t_loc, H, NCH * 128), np.float32)
    qpad[:, :, :D] = qs
    qtr = qpad.reshape(t_loc, H, NCH, 128).transpose(0, 3, 2, 1)
    if mode == "bf3":
        import ml_dtypes
        q_hi = qtr.astype(ml_dtypes.bfloat16)
        q_lo = (qtr - q_hi.astype(np.float32)).astype(ml_dtypes.bfloat16)
        # [t, 128, NCH, 2, H]
        q_t = np.ascontiguousarray(
            np.stack([q_hi, q_lo], axis=3))
    else:
        q_t = np.ascontiguousarray(qtr)

    tk = np.asarray(topk_indices[t0:t0 + t_loc]).astype(np.int16)
    wrap = tk.reshape(t_loc, TOPK // 16, 16).transpose(0, 2, 1)
    idx = np.ascontiguousarray(np.tile(wrap, (1, 8, 1)))

    return {"q_t": q_t, "kv": kv_rep, "idx": idx, "esink": esink,
            "ident": ident}


_PROGRAM_CACHE = {}


def _get_program(t_loc, mode=MODE):
    key = (t_loc, mode)
    if key not in _PROGRAM_CACHE:
        _PROGRAM_CACHE[key] = build_program(t_loc, mode=mode)
    return _PROGRAM_CACHE[key]


def run(q, kv_cache, topk_indices, attn_sink, trace=False, mode=MODE):
    nc = _get_program(T_LOC, mode)
    kv_rep = np.ascontiguousarray(np.asarray(kv_cache, np.float32))
    if mode in ("pv_r", "all_r"):
        kv_rep = round_f32r(kv_rep)
    es = np.exp(np.asarray(attn_sink, np.float64)).astype(np.float32)
    esink = np.ascontiguousarray(np.tile(es, 2)[:, None])
    ident = np.eye(128, dtype=np.float32)
    in_maps = [
        prep_core_inputs(q, kv_rep, topk_indices, esink, ident, c, mode=mode)
        for c in range(N_CORES)
    ]
    res = run_bass_kernel_spmd(nc, in_maps, list(range(N_CORES)),
                               trace=trace)
    out = np.concatenate([res.results[c]["out"] for c in range(N_CORES)],
                         axis=0)
    return out, res


def kernel(q, kv_cache, topk_indices, attn_sink):
    out, _ = run(q, kv_cache, topk_indices, attn_sink, trace=False)
    return out.astype(np.float32)



# revision 3
# speedup vs baseline: 6.6980x; 6.6980x over previous
"""DeepSeek-V4 MLA sparse attention — Trainium2 Bass kernel, 8 NeuronCores.

Contract: kernel(**inputs) takes the FULL unsharded inputs
  q [512,64,576] f32, kv_cache [32768,576] f32,
  topk_indices [512,512] i32, attn_sink [64] f32
and returns the FULL output [512,64,512] f32.

Strategy (token/data-parallel per the sharding hint):
  - tokens sharded 8 ways (64/core); kv_cache replicated per core.

MODE "f16" (default, rel err ~1e-3 << 2e-2 gate):
  - KV cache host-converted to fp16 and padded to 640 cols (1280B rows,
    %256 for the DGE). q host-scaled, zero-padded to 640, d-major fp16
    [t,128p,5c,64h]; topk -> int16 SWDGE wrap.
  - device, per token pair (A,B):
      * gpsimd dma_gather(transpose=True): K^T lands directly in SBUF as
        [128 (d%128), 5 (d//128), 512 j] fp16 — no PE transposes and no
        PSUM drains to build K^T (the fp32 baseline couldn't use this:
        transpose-mode gather requires <=16-bit dtype).
      * V ([j, dv] layout for the PV rhs) rebuilt from K^T's latent
        chunks by 16 PE transposes per token; PSUM->SBUF drains split
        between ScalarE and DVE.
      * QK^T: fp16 matmuls column-tiled (token A -> psum partitions
        0-63, B -> 64-127), accumulating over the 5 d-chunks (zero-pad
        makes all chunks full 128 rows).
      * sink-softmax with NO max-subtraction (shift-invariant; logits
        ~N(0,1)): p = exp(s) fp16 + fp32 row-sum in one ScalarE op;
        denom/recip on DVE.
      * p^T via one [128,128] PE transpose per topk block.
      * PV: fp16 matmuls column-tiled over 4 topk blocks; out = pv *
        (1/denom) fused with the PSUM drain on DVE; fp32 store.

MODE "fp32": the original all-fp32 kernel (rel err ~3e-6), kept as a
fallback; normal (non-transposed) gather + PE-transposed K^T.
"""

import numpy as np
from contextlib import ExitStack, nullcontext

import concourse.mybir as mybir
import concourse.tile as tile
from concourse import bacc
from concourse.bass_utils import run_bass_kernel_spmd

F32 = mybir.dt.float32
F16 = mybir.dt.float16
I16 = mybir.dt.int16

T_FULL = 512
H = 64
D = 576
DV = 512
NKV = 32768
TOPK = 512
N_CORES = 8
T_LOC = T_FULL // N_CORES
SCALE = float(D) ** -0.5
NCH = 5       # d-chunks of 128 (576 zero-padded to 640)
DP = NCH * 128  # 640: padded row length (1280B fp16, %256 for the DGE)
NB = TOPK // 128  # topk blocks of 128

MODE = "f16"


def build_program_f16(t_loc=T_LOC, repeat=1):
    nc = bacc.Bacc("TRN2", target_bir_lowering=False, debug=False,
                   num_swdge_queues=2)
    q_t = nc.dram_tensor("q_t", [t_loc, 128, NCH, H], F16,
                         kind="ExternalInput")
    kv = nc.dram_tensor("kv", [NKV, DP], F16, kind="ExternalInput")
    idx = nc.dram_tensor("idx", [t_loc, 128, TOPK // 16], I16,
                         kind="ExternalInput")
    esink = nc.dram_tensor("esink", [128, 1], F32, kind="ExternalInput")
    ident_d = nc.dram_tensor("ident", [128, 128], F16, kind="ExternalInput")
    out = nc.dram_tensor("out", [t_loc, H, DV], F32, kind="ExternalOutput")

    out_flat = out.ap().rearrange("t h d -> (t h) d")

    with tile.TileContext(nc) as tc, ExitStack() as ctx:
        consts = ctx.enter_context(tc.tile_pool(name="consts", bufs=1))
        kq = ctx.enter_context(tc.tile_pool(name="kq", bufs=4))
        vp = ctx.enter_context(tc.tile_pool(name="vp", bufs=3))
        soft = ctx.enter_context(tc.tile_pool(name="soft", bufs=2))
        outp = ctx.enter_context(tc.tile_pool(name="outp", bufs=2))
        small = ctx.enter_context(tc.tile_pool(name="small", bufs=4))
        ps_vt = ctx.enter_context(
            tc.tile_pool(name="ps_vt", bufs=2, space="PSUM"))
        ps_sc = ctx.enter_context(
            tc.tile_pool(name="ps_sc", bufs=2, space="PSUM"))
        ps_pt = ctx.enter_context(
            tc.tile_pool(name="ps_pt", bufs=2, space="PSUM"))
        ps_pv = ctx.enter_context(
            tc.tile_pool(name="ps_pv", bufs=2, space="PSUM"))

        ident = consts.tile([128, 128], F16)
        nc.sync.dma_start(out=ident[:], in_=ident_d.ap())
        es_sb = consts.tile([128, 1], F32)
        nc.sync.dma_start(out=es_sb[:], in_=esink.ap())
        # Warmup transpose absorbs the identity-DMA wait up front.
        warm = ps_pt.tile([128, 128], F16, tag="ps_pt")
        nc.tensor.transpose(warm[:], ident[:], ident[:])

        def load_token(t, queue):
            idx_sb = kq.tile([128, TOPK // 16], I16, tag="idx")
            nc.sync.dma_start(out=idx_sb[:], in_=idx.ap()[t])
            # K^T gathered directly: out[p, c, j] = kv[idx[j], c*128+p]
            kt_sb = kq.tile([128, NCH, TOPK], F16, tag="kt")
            nc.gpsimd.dma_gather(
                out_ap=kt_sb[:],
                in_ap=kv.ap(),
                idxs_ap=idx_sb[:],
                num_idxs=TOPK,
                num_idxs_reg=TOPK,
                elem_size=DP,
                transpose=True,
                queue_num=queue,
            )
            q_sb = kq.tile([128, NCH, H], F16, tag="q")
            nc.sync.dma_start(out=q_sb[:], in_=q_t.ap()[t])
            return kt_sb, q_sb

        def build_v(kt_sb, tok):
            # V[j, dv] from the latent chunks of K^T via PE transposes.
            v_sb = vp.tile([128, NB, DV], F16, tag="v")
            for b in range(NB):
                vps = ps_vt.tile([128, DV], F16, tag="ps_vt")
                for c in range(4):
                    nc.tensor.transpose(
                        vps[:, c * 128:(c + 1) * 128],
                        kt_sb[:, c, b * 128:(b + 1) * 128],
                        ident[:],
                    )
                if (tok * NB + b) % 2 == 0:
                    nc.scalar.copy(v_sb[:, b, :], vps[:])
                else:
                    nc.vector.tensor_copy(v_sb[:, b, :], vps[:])
            return v_sb

        def pair_body(tA):
            ktA, qA = load_token(tA, 0)
            ktB, qB = load_token(tA + 1, 1)
            vA = build_v(ktA, 0)

            sc = ps_sc.tile([128, TOPK], F32, tag="sc")
            for c in range(NCH):
                st, sp = (c == 0), (c == NCH - 1)
                nc.tensor.matmul(
                    sc[0:64, :], lhsT=qA[:, c, :], rhs=ktA[:, c, :],
                    start=st, stop=sp, tile_position=(0, 0),
                    skip_group_check=True,
                )
                nc.tensor.matmul(
                    sc[64:128, :], lhsT=qB[:, c, :], rhs=ktB[:, c, :],
                    start=st, stop=sp, tile_position=(0, 64),
                    skip_group_check=True,
                )

            vB = build_v(ktB, 1)

            p_sb = soft.tile([128, TOPK], F16, tag="p")
            sum_p = small.tile([128, 1], F32, tag="sum")
            nc.scalar.activation(
                p_sb[:], sc[:], mybir.ActivationFunctionType.Exp,
                accum_out=sum_p[:],
            )
            den = small.tile([128, 1], F32, tag="den")
            nc.vector.tensor_add(den[:], sum_p[:], es_sb[:])
            rec = small.tile([128, 1], F32, tag="rec")
            nc.vector.reciprocal(rec[:], den[:])

            pt_sb = soft.tile([128, NB, 128], F16, tag="pt")
            for b in range(NB):
                pst = ps_pt.tile([128, 128], F16, tag="ps_pt")
                nc.tensor.transpose(
                    pst[:], p_sb[:, b * 128:(b + 1) * 128], ident[:])
                nc.vector.tensor_copy(pt_sb[:, b, :], pst[:])

            pv = ps_pv.tile([128, DV], F32, tag="pv")
            for b in range(NB):
                st, sp = (b == 0), (b == NB - 1)
                nc.tensor.matmul(
                    pv[0:64, :], lhsT=pt_sb[:, b, 0:64], rhs=vA[:, b, :],
                    start=st, stop=sp, tile_position=(0, 0),
                    skip_group_check=True,
                )
                nc.tensor.matmul(
                    pv[64:128, :], lhsT=pt_sb[:, b, 64:128], rhs=vB[:, b, :],
                    start=st, stop=sp, tile_position=(0, 64),
                    skip_group_check=True,
                )

            o_sb = outp.tile([128, DV], F32, tag="o")
            nc.vector.tensor_scalar_mul(o_sb[:], pv[:], rec[:])
            nc.sync.dma_start(
                out=out_flat[tA * H:tA * H + 128, :], in_=o_sb[:])

        loop_cm = tc.For_i(0, repeat, 1) if repeat > 1 else nullcontext()
        with loop_cm:
            for i in range(t_loc // 2):
                pair_body(2 * i)

    nc.compile()
    return nc


def build_program_fp32(t_loc=T_LOC, repeat=1):
    """Original all-fp32 kernel (rel err ~3e-6); normal gather + PE K^T."""
    nc = bacc.Bacc("TRN2", target_bir_lowering=False, debug=False)
    q_t = nc.dram_tensor("q_t", [t_loc, 128, NCH, H], F32,
                         kind="ExternalInput")
    kv = nc.dram_tensor("kv", [NKV, D], F32, kind="ExternalInput")
    idx = nc.dram_tensor("idx", [t_loc, 128, TOPK // 16], I16,
                         kind="ExternalInput")
    esink = nc.dram_tensor("esink", [128, 1], F32, kind="ExternalInput")
    ident_d = nc.dram_tensor("ident", [128, 128], F32, kind="ExternalInput")
    out = nc.dram_tensor("out", [t_loc, H, DV], F32, kind="ExternalOutput")

    out_flat = out.ap().rearrange("t h d -> (t h) d")

    with tile.TileContext(nc) as tc, ExitStack() as ctx:
        consts = ctx.enter_context(tc.tile_pool(name="consts", bufs=1))
        kq = ctx.enter_context(tc.tile_pool(name="kq", bufs=5))
        ktp = ctx.enter_context(tc.tile_pool(name="ktp", bufs=3))
        soft = ctx.enter_context(tc.tile_pool(name="soft", bufs=2))
        outp = ctx.enter_context(tc.tile_pool(name="outp", bufs=2))
        small = ctx.enter_context(tc.tile_pool(name="small", bufs=4))
        ps_kt = ctx.enter_context(
            tc.tile_pool(name="ps_kt", bufs=2, space="PSUM"))
        ps_sc = ctx.enter_context(
            tc.tile_pool(name="ps_sc", bufs=2, space="PSUM"))
        ps_pt = ctx.enter_context(
            tc.tile_pool(name="ps_pt", bufs=2, space="PSUM"))
        ps_pv = ctx.enter_context(
            tc.tile_pool(name="ps_pv", bufs=2, space="PSUM"))

        ident = consts.tile([128, 128], F32)
        nc.sync.dma_start(out=ident[:], in_=ident_d.ap())
        es_sb = consts.tile([128, 1], F32)
        nc.sync.dma_start(out=es_sb[:], in_=esink.ap())
        warm = ps_pt.tile([128, 128], F32, tag="ps_pt")
        nc.tensor.transpose(warm[:], ident[:], ident[:])

        def load_token(t):
            idx_sb = kq.tile([128, TOPK // 16], I16, tag="idx")
            nc.sync.dma_start(out=idx_sb[:], in_=idx.ap()[t])
            k_sb = kq.tile([128, NB, D], F32, tag="k")
            nc.gpsimd.dma_gather(
                out_ap=k_sb[:],
                in_ap=kv.ap(),
                idxs_ap=idx_sb[:],
                num_idxs=TOPK,
                num_idxs_reg=TOPK,
                elem_size=D,
            )
            q_sb = kq.tile([128, NCH, H], F32, tag="q")
            nc.sync.dma_start(out=q_sb[:], in_=q_t.ap()[t])
            q_act = kq.tile([128, NCH, H], F32, tag="qa")
            nc.scalar.copy(q_act[:], q_sb[:])
            return k_sb, q_act

        def build_kt(k_sb):
            kt_sb = ktp.tile([128, NCH, TOPK], F32, tag="kt")
            for c in range(NCH):
                pp = 128 if c < 4 else D - 512
                pst = ps_kt.tile([128, TOPK], F32, tag="ps_kt")
                for b in range(NB):
                    nc.tensor.transpose(
                        pst[:pp, b * 128:(b + 1) * 128],
                        k_sb[:, b, c * 128:c * 128 + pp],
                        ident[:],
                    )
                nc.scalar.copy(kt_sb[:pp, c, :], pst[:pp, :])
            return kt_sb

        def pair_body(tA):
            kA, qA = load_token(tA)
            kB, qB = load_token(tA + 1)
            ktA = build_kt(kA)
            ktB = build_kt(kB)

            sc = ps_sc.tile([128, TOPK], F32, tag="sc")
            for c in range(NCH):
                kk = 128 if c < 4 else D - 512
                st, sp = (c == 0), (c == NCH - 1)
                nc.tensor.matmul(
                    sc[0:64, :], lhsT=qA[:kk, c, :], rhs=ktA[:kk, c, :],
                    start=st, stop=sp, tile_position=(0, 0),
                    skip_group_check=True,
                )
                nc.tensor.matmul(
                    sc[64:128, :], lhsT=qB[:kk, c, :], rhs=ktB[:kk, c, :],
                    start=st, stop=sp, tile_position=(0, 64),
                    skip_group_check=True,
                )

            p_sb = soft.tile([128, TOPK], F32, tag="p")
            sum_p = small.tile([128, 1], F32, tag="sum")
            nc.scalar.activation(
                p_sb[:], sc[:], mybir.ActivationFunctionType.Exp,
                accum_out=sum_p[:],
            )
            den = small.tile([128, 1], F32, tag="den")
            nc.vector.tensor_add(den[:], sum_p[:], es_sb[:])
            rec = small.tile([128, 1], F32, tag="rec")
            nc.vector.reciprocal(rec[:], den[:])

            pt_sb = soft.tile([128, NB, 128], F32, tag="pt")
            for b in range(NB):
                pst = ps_pt.tile([128, 128], F32, tag="ps_pt")
                nc.tensor.transpose(
                    pst[:], p_sb[:, b * 128:(b + 1) * 128], ident[:])
                nc.vector.tensor_copy(pt_sb[:, b, :], pst[:])

            pv = ps_pv.tile([128, DV], F32, tag="pv")
            for b in range(NB):
                st, sp = (b == 0), (b == NB - 1)
                nc.tensor.matmul(
                    pv[0:64, :], lhsT=pt_sb[:, b, 0:64], rhs=kA[:, b, 0:DV],
                    start=st, stop=sp, tile_position=(0, 0),
                    skip_group_check=True,
                )
                nc.tensor.matmul(
                    pv[64:128, :], lhsT=pt_sb[:, b, 64:128],
                    rhs=kB[:, b, 0:DV],
                    start=st, stop=sp, tile_position=(0, 64),
                    skip_group_check=True,
                )

            o_sb = outp.tile([128, DV], F32, tag="o")
            nc.vector.tensor_scalar_mul(o_sb[:], pv[:], rec[:])
            nc.sync.dma_start(
                out=out_flat[tA * H:tA * H + 128, :], in_=o_sb[:])

        loop_cm = tc.For_i(0, repeat, 1) if repeat > 1 else nullcontext()
        with loop_cm:
            for i in range(t_loc // 2):
                pair_body(2 * i)

    nc.compile()
    return nc


def build_program(t_loc=T_LOC, repeat=1, mode=MODE):
    if mode == "f16":
        return build_program_f16(t_loc, repeat)
    assert mode == "fp32"
    return build_program_fp32(t_loc, repeat)


# ---------------- host-side prep ----------------

def prep_shared(kv_cache, attn_sink, mode=MODE):
    """Per-run, core-independent host prep (replicated to every core)."""
    es = np.exp(np.asarray(attn_sink, np.float64)).astype(np.float32)
    esink = np.ascontiguousarray(np.tile(es, 2)[:, None])
    if mode == "f16":
        kv = np.zeros((NKV, DP), np.float16)
        kv[:, :D] = np.asarray(kv_cache, np.float32).astype(np.float16)
        ident = np.eye(128, dtype=np.float16)
    else:
        kv = np.ascontiguousarray(np.asarray(kv_cache, np.float32))
        ident = np.eye(128, dtype=np.float32)
    return {"kv": kv, "esink": esink, "ident": ident}


def prep_core_inputs(q, shared, topk_indices, core, t_loc=T_LOC, mode=MODE):
    t0 = core * t_loc
    qs = (np.asarray(q[t0:t0 + t_loc]) * SCALE).astype(np.float32)
    qpad = np.zeros((t_loc, H, DP), np.float32)
    qpad[:, :, :D] = qs
    qtr = qpad.reshape(t_loc, H, NCH, 128).transpose(0, 3, 2, 1)
    if mode == "f16":
        q_t = np.ascontiguousarray(qtr.astype(np.float16))
    else:
        q_t = np.ascontiguousarray(qtr)

    tk = np.asarray(topk_indices[t0:t0 + t_loc]).astype(np.int16)
    wrap = tk.reshape(t_loc, TOPK // 16, 16).transpose(0, 2, 1)
    idx = np.ascontiguousarray(np.tile(wrap, (1, 8, 1)))

    return {"q_t": q_t, "kv": shared["kv"], "idx": idx,
            "esink": shared["esink"], "ident": shared["ident"]}


_PROGRAM_CACHE = {}


def _get_program(t_loc, mode=MODE):
    key = (t_loc, mode)
    if key not in _PROGRAM_CACHE:
        _PROGRAM_CACHE[key] = build_program(t_loc, mode=mode)
    return _PROGRAM_CACHE[key]


def run(q, kv_cache, topk_indices, attn_sink, trace=False, mode=MODE):
    nc = _get_program(T_LOC, mode)
    shared = prep_shared(kv_cache, attn_sink, mode)
    in_maps = [
        prep_core_inputs(q, shared, topk_indices, c, mode=mode)
        for c in range(N_CORES)
    ]
    res = run_bass_kernel_spmd(nc, in_maps, list(range(N_CORES)),
                               trace=trace)
    out = np.concatenate([res.results[c]["out"] for c in range(N_CORES)],
                         axis=0)
    return out, res


def kernel(q, kv_cache, topk_indices, attn_sink):
    out, _ = run(q, kv_cache, topk_indices, attn_sink, trace=False)
    return out.astype(np.float32)


# revision 9
# speedup vs baseline: 8.6514x; 1.2916x over previous
"""DeepSeek-V4 MLA sparse attention — Trainium2 Bass kernel, 8 NeuronCores.

Contract: kernel(**inputs) takes the FULL unsharded inputs
  q [512,64,576] f32, kv_cache [32768,576] f32,
  topk_indices [512,512] i32, attn_sink [64] f32
and returns the FULL output [512,64,512] f32.

Strategy (token/data-parallel per the sharding hint):
  - tokens sharded 8 ways (64/core); kv_cache replicated per core.

MODE "f16" (default, rel err ~1e-3 << 2e-2 gate):
  - KV cache host-converted to fp16 and padded to 640 cols (1280B rows,
    %256 for the DGE). q host-scaled, zero-padded to 640, d-major fp16
    [t,128p,5c,64h]; topk -> int16 SWDGE wrap.
  - device, per token pair (A,B):
      * gpsimd dma_gather(transpose=True): K^T lands directly in SBUF as
        [128 (d%128), 5 (d//128), 512 j] fp16 — no PE transposes and no
        PSUM drains to build K^T (the fp32 baseline couldn't use this:
        transpose-mode gather requires <=16-bit dtype).
      * V ([j, dv] layout for the PV rhs) rebuilt from K^T's latent
        chunks by 16 PE transposes per token; PSUM->SBUF drains split
        between ScalarE and DVE.
      * QK^T: fp16 matmuls column-tiled (token A -> psum partitions
        0-63, B -> 64-127), accumulating over the 5 d-chunks (zero-pad
        makes all chunks full 128 rows).
      * sink-softmax with NO max-subtraction (shift-invariant; logits
        ~N(0,1)): p = exp(s) fp16 + fp32 row-sum in one ScalarE op;
        denom/recip on DVE.
      * p^T via one [128,128] PE transpose per topk block.
      * PV: fp16 matmuls column-tiled over 4 topk blocks; out = pv *
        (1/denom) fused with the PSUM drain on DVE; fp32 store.

MODE "fp32": the original all-fp32 kernel (rel err ~3e-6), kept as a
fallback; normal (non-transposed) gather + PE-transposed K^T.
"""

import numpy as np
from contextlib import ExitStack, nullcontext

import concourse.mybir as mybir
import concourse.tile as tile
from concourse import bacc
from concourse.bass_utils import run_bass_kernel_spmd

F32 = mybir.dt.float32
F16 = mybir.dt.float16
I16 = mybir.dt.int16

T_FULL = 512
H = 64
D = 576
DV = 512
NKV = 32768
TOPK = 512
N_CORES = 8
T_LOC = T_FULL // N_CORES
SCALE = float(D) ** -0.5
NCH = 5       # d-chunks of 128 (576 zero-padded to 640)
DP = NCH * 128  # 640: padded row length (1280B fp16, %256 for the DGE)
NB = TOPK // 128  # topk blocks of 128

MODE = "f16"


def build_program_f16(t_loc=T_LOC, repeat=1, ablate=None, single_packet=True,
                      n_queues=2):
    nc = bacc.Bacc("TRN2", target_bir_lowering=False, debug=False,
                   num_swdge_queues=n_queues)
    q_t = nc.dram_tensor("q_t", [t_loc, 128, NCH, H], F16,
                         kind="ExternalInput")
    kv = nc.dram_tensor("kv", [NKV, DP], F16, kind="ExternalInput")
    idx = nc.dram_tensor("idx", [t_loc, 128, TOPK // 16], I16,
                         kind="ExternalInput")
    esink = nc.dram_tensor("esink", [128, 1], F32, kind="ExternalInput")
    ident_d = nc.dram_tensor("ident", [128, 128], F16, kind="ExternalInput")
    out = nc.dram_tensor("out", [t_loc, H, DV], F32, kind="ExternalOutput")

    out_flat = out.ap().rearrange("t h d -> (t h) d")

    with tile.TileContext(nc) as tc, ExitStack() as ctx:
        consts = ctx.enter_context(tc.tile_pool(name="consts", bufs=1))
        kq = ctx.enter_context(tc.tile_pool(name="kq", bufs=4))
        vp = ctx.enter_context(tc.tile_pool(name="vp", bufs=3))
        soft = ctx.enter_context(tc.tile_pool(name="soft", bufs=2))
        outp = ctx.enter_context(tc.tile_pool(name="outp", bufs=2))
        small = ctx.enter_context(tc.tile_pool(name="small", bufs=4))
        ps_vt = ctx.enter_context(
            tc.tile_pool(name="ps_vt", bufs=2, space="PSUM"))
        ps_sc = ctx.enter_context(
            tc.tile_pool(name="ps_sc", bufs=2, space="PSUM"))
        ps_pt = ctx.enter_context(
            tc.tile_pool(name="ps_pt", bufs=2, space="PSUM"))
        ps_pv = ctx.enter_context(
            tc.tile_pool(name="ps_pv", bufs=2, space="PSUM"))

        ident = consts.tile([128, 128], F16)
        nc.sync.dma_start(out=ident[:], in_=ident_d.ap())
        es_sb = consts.tile([128, 1], F32)
        nc.sync.dma_start(out=es_sb[:], in_=esink.ap())
        # Warmup transpose absorbs the identity-DMA wait up front.
        warm = ps_pt.tile([128, 128], F16, tag="ps_pt")
        nc.tensor.transpose(warm[:], ident[:], ident[:])
        if ablate == "gather":
            junk = consts.tile([128, DV], F32)
            for cc in range(4):
                nc.vector.tensor_copy(junk[:, cc * 128:(cc + 1) * 128],
                                      ident[:])

        def load_token(t, queue):
            idx_sb = kq.tile([128, TOPK // 16], I16, tag="idx")
            nc.sync.dma_start(out=idx_sb[:], in_=idx.ap()[t])
            # K^T gathered directly: out[p, c, j] = kv[idx[j], c*128+p]
            kt_sb = kq.tile([128, NCH, TOPK], F16, tag="kt")
            if ablate == "compute":
                pass  # no gather: compute-only floor (kt_sb is garbage)
            elif n_queues == 4:
                half = TOPK // 2
                for h_i in range(2):
                    nc.gpsimd.dma_gather(
                        out_ap=kt_sb[:, :, h_i * half:(h_i + 1) * half],
                        in_ap=kv.ap(),
                        idxs_ap=idx_sb[:, h_i * (half // 16):
                                       (h_i + 1) * (half // 16)],
                        num_idxs=half,
                        num_idxs_reg=half,
                        elem_size=DP,
                        transpose=True,
                        single_packet=single_packet,
                        queue_num=2 * queue + h_i,
                    )
            else:
                nc.gpsimd.dma_gather(
                    out_ap=kt_sb[:],
                    in_ap=kv.ap(),
                    idxs_ap=idx_sb[:],
                    num_idxs=TOPK,
                    num_idxs_reg=TOPK,
                    elem_size=DP,
                    transpose=True,
                    single_packet=single_packet,
                    queue_num=queue,
                )
            q_sb = kq.tile([128, NCH, H], F16, tag="q")
            nc.sync.dma_start(out=q_sb[:], in_=q_t.ap()[t])
            return kt_sb, q_sb

        def build_v(kt_sb, tok):
            # V[j, dv] from the latent chunks of K^T via PE transposes.
            v_sb = vp.tile([128, NB, DV], F16, tag="v")
            for b in range(NB):
                vps = ps_vt.tile([128, DV], F16, tag="ps_vt")
                for c in range(4):
                    nc.tensor.transpose(
                        vps[:, c * 128:(c + 1) * 128],
                        kt_sb[:, c, b * 128:(b + 1) * 128],
                        ident[:],
                    )
                if (tok * NB + b) % 2 == 0:
                    nc.scalar.copy(v_sb[:, b, :], vps[:])
                else:
                    nc.vector.tensor_copy(v_sb[:, b, :], vps[:])
            return v_sb

        def pair_body(tA):
            ktA, qA = load_token(tA, 0)
            ktB, qB = load_token(tA + 1, 1)
            if ablate == "gather":
                # DMA-only floor: loads + a junk store, no compute.
                nc.sync.dma_start(
                    out=out_flat[tA * H:tA * H + 128, :], in_=junk[:])
                return
            vA = build_v(ktA, 0)

            sc = ps_sc.tile([128, TOPK], F32, tag="sc")
            for c in range(NCH):
                st, sp = (c == 0), (c == NCH - 1)
                nc.tensor.matmul(
                    sc[0:64, :], lhsT=qA[:, c, :], rhs=ktA[:, c, :],
                    start=st, stop=sp, tile_position=(0, 0),
                    skip_group_check=True,
                )
                nc.tensor.matmul(
                    sc[64:128, :], lhsT=qB[:, c, :], rhs=ktB[:, c, :],
                    start=st, stop=sp, tile_position=(0, 64),
                    skip_group_check=True,
                )

            vB = build_v(ktB, 1)

            p_sb = soft.tile([128, TOPK], F16, tag="p")
            sum_p = small.tile([128, 1], F32, tag="sum")
            nc.scalar.activation(
                p_sb[:], sc[:], mybir.ActivationFunctionType.Exp,
                accum_out=sum_p[:],
            )
            den = small.tile([128, 1], F32, tag="den")
            nc.vector.tensor_add(den[:], sum_p[:], es_sb[:])
            rec = small.tile([128, 1], F32, tag="rec")
            nc.vector.reciprocal(rec[:], den[:])

            pt_sb = soft.tile([128, NB, 128], F16, tag="pt")
            for b in range(NB):
                pst = ps_pt.tile([128, 128], F16, tag="ps_pt")
                nc.tensor.transpose(
                    pst[:], p_sb[:, b * 128:(b + 1) * 128], ident[:])
                nc.vector.tensor_copy(pt_sb[:, b, :], pst[:])

            pv = ps_pv.tile([128, DV], F32, tag="pv")
            for b in range(NB):
                st, sp = (b == 0), (b == NB - 1)
                nc.tensor.matmul(
                    pv[0:64, :], lhsT=pt_sb[:, b, 0:64], rhs=vA[:, b, :],
                    start=st, stop=sp, tile_position=(0, 0),
                    skip_group_check=True,
                )
                nc.tensor.matmul(
                    pv[64:128, :], lhsT=pt_sb[:, b, 64:128], rhs=vB[:, b, :],
                    start=st, stop=sp, tile_position=(0, 64),
                    skip_group_check=True,
                )

            o_sb = outp.tile([128, DV], F32, tag="o")
            nc.vector.tensor_scalar_mul(o_sb[:], pv[:], rec[:])
            nc.sync.dma_start(
                out=out_flat[tA * H:tA * H + 128, :], in_=o_sb[:])

        loop_cm = tc.For_i(0, repeat, 1) if repeat > 1 else nullcontext()
        with loop_cm:
            for i in range(t_loc // 2):
                pair_body(2 * i)

    nc.compile()
    return nc


def build_program_fp32(t_loc=T_LOC, repeat=1):
    """Original all-fp32 kernel (rel err ~3e-6); normal gather + PE K^T."""
    nc = bacc.Bacc("TRN2", target_bir_lowering=False, debug=False)
    q_t = nc.dram_tensor("q_t", [t_loc, 128, NCH, H], F32,
                         kind="ExternalInput")
    kv = nc.dram_tensor("kv", [NKV, D], F32, kind="ExternalInput")
    idx = nc.dram_tensor("idx", [t_loc, 128, TOPK // 16], I16,
                         kind="ExternalInput")
    esink = nc.dram_tensor("esink", [128, 1], F32, kind="ExternalInput")
    ident_d = nc.dram_tensor("ident", [128, 128], F32, kind="ExternalInput")
    out = nc.dram_tensor("out", [t_loc, H, DV], F32, kind="ExternalOutput")

    out_flat = out.ap().rearrange("t h d -> (t h) d")

    with tile.TileContext(nc) as tc, ExitStack() as ctx:
        consts = ctx.enter_context(tc.tile_pool(name="consts", bufs=1))
        kq = ctx.enter_context(tc.tile_pool(name="kq", bufs=5))
        ktp = ctx.enter_context(tc.tile_pool(name="ktp", bufs=3))
        soft = ctx.enter_context(tc.tile_pool(name="soft", bufs=2))
        outp = ctx.enter_context(tc.tile_pool(name="outp", bufs=2))
        small = ctx.enter_context(tc.tile_pool(name="small", bufs=4))
        ps_kt = ctx.enter_context(
            tc.tile_pool(name="ps_kt", bufs=2, space="PSUM"))
        ps_sc = ctx.enter_context(
            tc.tile_pool(name="ps_sc", bufs=2, space="PSUM"))
        ps_pt = ctx.enter_context(
            tc.tile_pool(name="ps_pt", bufs=2, space="PSUM"))
        ps_pv = ctx.enter_context(
            tc.tile_pool(name="ps_pv", bufs=2, space="PSUM"))

        ident = consts.tile([128, 128], F32)
        nc.sync.dma_start(out=ident[:], in_=ident_d.ap())
        es_sb = consts.tile([128, 1], F32)
        nc.sync.dma_start(out=es_sb[:], in_=esink.ap())
        warm = ps_pt.tile([128, 128], F32, tag="ps_pt")
        nc.tensor.transpose(warm[:], ident[:], ident[:])

        def load_token(t):
            idx_sb = kq.tile([128, TOPK // 16], I16, tag="idx")
            nc.sync.dma_start(out=idx_sb[:], in_=idx.ap()[t])
            k_sb = kq.tile([128, NB, D], F32, tag="k")
            nc.gpsimd.dma_gather(
                out_ap=k_sb[:],
                in_ap=kv.ap(),
                idxs_ap=idx_sb[:],
                num_idxs=TOPK,
                num_idxs_reg=TOPK,
                elem_size=D,
            )
            q_sb = kq.tile([128, NCH, H], F32, tag="q")
            nc.sync.dma_start(out=q_sb[:], in_=q_t.ap()[t])
            q_act = kq.tile([128, NCH, H], F32, tag="qa")
            nc.scalar.copy(q_act[:], q_sb[:])
            return k_sb, q_act

        def build_kt(k_sb):
            kt_sb = ktp.tile([128, NCH, TOPK], F32, tag="kt")
            for c in range(NCH):
                pp = 128 if c < 4 else D - 512
                pst = ps_kt.tile([128, TOPK], F32, tag="ps_kt")
                for b in range(NB):
                    nc.tensor.transpose(
                        pst[:pp, b * 128:(b + 1) * 128],
                        k_sb[:, b, c * 128:c * 128 + pp],
                        ident[:],
                    )
                nc.scalar.copy(kt_sb[:pp, c, :], pst[:pp, :])
            return kt_sb

        def pair_body(tA):
            kA, qA = load_token(tA)
            kB, qB = load_token(tA + 1)
            ktA = build_kt(kA)
            ktB = build_kt(kB)

            sc = ps_sc.tile([128, TOPK], F32, tag="sc")
            for c in range(NCH):
                kk = 128 if c < 4 else D - 512
                st, sp = (c == 0), (c == NCH - 1)
                nc.tensor.matmul(
                    sc[0:64, :], lhsT=qA[:kk, c, :], rhs=ktA[:kk, c, :],
                    start=st, stop=sp, tile_position=(0, 0),
                    skip_group_check=True,
                )
                nc.tensor.matmul(
                    sc[64:128, :], lhsT=qB[:kk, c, :], rhs=ktB[:kk, c, :],
                    start=st, stop=sp, tile_position=(0, 64),
                    skip_group_check=True,
                )

            p_sb = soft.tile([128, TOPK], F32, tag="p")
            sum_p = small.tile([128, 1], F32, tag="sum")
            nc.scalar.activation(
                p_sb[:], sc[:], mybir.ActivationFunctionType.Exp,
                accum_out=sum_p[:],
            )
            den = small.tile([128, 1], F32, tag="den")
            nc.vector.tensor_add(den[:], sum_p[:], es_sb[:])
            rec = small.tile([128, 1], F32, tag="rec")
            nc.vector.reciprocal(rec[:], den[:])

            pt_sb = soft.tile([128, NB, 128], F32, tag="pt")
            for b in range(NB):
                pst = ps_pt.tile([128, 128], F32, tag="ps_pt")
                nc.tensor.transpose(
                    pst[:], p_sb[:, b * 128:(b + 1) * 128], ident[:])
                nc.vector.tensor_copy(pt_sb[:, b, :], pst[:])

            pv = ps_pv.tile([128, DV], F32, tag="pv")
            for b in range(NB):
                st, sp = (b == 0), (b == NB - 1)
                nc.tensor.matmul(
                    pv[0:64, :], lhsT=pt_sb[:, b, 0:64], rhs=kA[:, b, 0:DV],
                    start=st, stop=sp, tile_position=(0, 0),
                    skip_group_check=True,
                )
                nc.tensor.matmul(
                    pv[64:128, :], lhsT=pt_sb[:, b, 64:128],
                    rhs=kB[:, b, 0:DV],
                    start=st, stop=sp, tile_position=(0, 64),
                    skip_group_check=True,
                )

            o_sb = outp.tile([128, DV], F32, tag="o")
            nc.vector.tensor_scalar_mul(o_sb[:], pv[:], rec[:])
            nc.sync.dma_start(
                out=out_flat[tA * H:tA * H + 128, :], in_=o_sb[:])

        loop_cm = tc.For_i(0, repeat, 1) if repeat > 1 else nullcontext()
        with loop_cm:
            for i in range(t_loc // 2):
                pair_body(2 * i)

    nc.compile()
    return nc


def build_program(t_loc=T_LOC, repeat=1, mode=MODE):
    if mode == "f16":
        return build_program_f16(t_loc, repeat)
    assert mode == "fp32"
    return build_program_fp32(t_loc, repeat)


# ---------------- host-side prep ----------------

def prep_shared(kv_cache, attn_sink, mode=MODE):
    """Per-run, core-independent host prep (replicated to every core)."""
    es = np.exp(np.asarray(attn_sink, np.float64)).astype(np.float32)
    esink = np.ascontiguousarray(np.tile(es, 2)[:, None])
    if mode == "f16":
        kv = np.zeros((NKV, DP), np.float16)
        kv[:, :D] = np.asarray(kv_cache, np.float32).astype(np.float16)
        ident = np.eye(128, dtype=np.float16)
    else:
        kv = np.ascontiguousarray(np.asarray(kv_cache, np.float32))
        ident = np.eye(128, dtype=np.float32)
    return {"kv": kv, "esink": esink, "ident": ident}


def prep_core_inputs(q, shared, topk_indices, core, t_loc=T_LOC, mode=MODE):
    t0 = core * t_loc
    qs = (np.asarray(q[t0:t0 + t_loc]) * SCALE).astype(np.float32)
    qpad = np.zeros((t_loc, H, DP), np.float32)
    qpad[:, :, :D] = qs
    qtr = qpad.reshape(t_loc, H, NCH, 128).transpose(0, 3, 2, 1)
    if mode == "f16":
        q_t = np.ascontiguousarray(qtr.astype(np.float16))
    else:
        q_t = np.ascontiguousarray(qtr)

    tk = np.asarray(topk_indices[t0:t0 + t_loc]).astype(np.int16)
    wrap = tk.reshape(t_loc, TOPK // 16, 16).transpose(0, 2, 1)
    idx = np.ascontiguousarray(np.tile(wrap, (1, 8, 1)))

    return {"q_t": q_t, "kv": shared["kv"], "idx": idx,
            "esink": shared["esink"], "ident": shared["ident"]}


_PROGRAM_CACHE = {}


def _get_program(t_loc, mode=MODE):
    key = (t_loc, mode)
    if key not in _PROGRAM_CACHE:
        _PROGRAM_CACHE[key] = build_program(t_loc, mode=mode)
    return _PROGRAM_CACHE[key]


def run(q, kv_cache, topk_indices, attn_sink, trace=False, mode=MODE):
    nc = _get_program(T_LOC, mode)
    shared = prep_shared(kv_cache, attn_sink, mode)
    in_maps = [
        prep_core_inputs(q, shared, topk_indices, c, mode=mode)
        for c in range(N_CORES)
    ]
    res = run_bass_kernel_spmd(nc, in_maps, list(range(N_CORES)),
                               trace=trace)
    out = np.concatenate([res.results[c]["out"] for c in range(N_CORES)],
                         axis=0)
    return out, res


def kernel(q, kv_cache, topk_indices, attn_sink):
    out, _ = run(q, kv_cache, topk_indices, attn_sink, trace=False)
    return out.astype(np.float32)
